# revision 24
# baseline (speedup 1.0000x reference)
"""Trainium2 Bass kernel for the DIP module (tone curve + white balance +
contrast-about-mean + 3x3 sharpen blend), data-parallel over batch on 8 cores.

v2 — fp16 end-to-end. Pipeline per (image, channel), channel = [128p, 4k, 512]:
  lt = Ln(x)                 ACT, one instr per IMAGE (FD=6144), fp16 in/out
  t' = a*wb * x^g            ACT Exp(scale=g, bias=ln(a*wb)) -> fp16,
                             accum_out = per-row sums (for the mean)
  cb = mean(t')*(1-a)/a      one fp32 matmul (const matrix) + DVE copy
  u  = clip01(t' + cb)       DVE tensor_scalar fp16->fp16 (4x mode); 1 op when
                             a<=1 (images permuted so slots sign-homogeneous)
  y  = clip01((1+8s)u - s*(8-neighbors))
                             4 fp16 matmuls per 128-row tile into PSUM with the
                             (1+8s) center folded into the mid matrix; halo rows
                             gathered by 2 DMAs, 3-summed on GPSIMD; final clip
                             reads 2-bank PSUM groups -> fp16 out, one DMA/image

I/O is fp16 in DRAM (host casts f32->fp16 on the way in, fp16->f32 out); loads
ride the sync HWDGE ring, stores + const the scalar ring, halo gathers on sync.
"""

import numpy as np

try:
    import concourse.bass as bass
except ImportError:  # pragma: no cover
    import sys

    sys.path.insert(0, "/opt/trn_rl_repo")
    import concourse.bass as bass

from contextlib import ExitStack

import concourse.bacc as bacc
import concourse.tile as tile
from concourse import mybir
from concourse.bass_utils import run_bass_kernel_spmd

F32 = mybir.dt.float32
F16 = mybir.dt.float16

B, C, H, W = 32, 3, 512, 512
NCORES = 8
IPC = B // NCORES  # images per core
NT = H // 128  # row tiles per channel
NPIX = H * W


class _Bacc(bacc.Bacc):
    """Bacc that pins Exp/Ln to the combined table set so the kernel does a
    single ACT_TABLE_LOAD instead of thrashing between exp/ln sets."""

    def insert_act_table_loads(self):
        import bass_rust as _bass_rust

        from concourse.hw_specs import get_activation_tables

        has_activation = any(
            isinstance(i, mybir.InstActivation)
            for b in self.main_func.blocks
            for i in b.instructions
        )
        if not has_activation:
            return
        AF = mybir.ActivationFunctionType
        tables = []
        for name, funcs in get_activation_tables(self.m.arch).items():
            if name != "natural_log_exp_and_others":
                funcs = funcs - {AF.Exp, AF.Ln}
            tables.append((name, funcs))
        _bass_rust.insert_act_table_loads(self, tables)


def _build_program(slotmask):
    nc = _Bacc("TRN2", target_bir_lowering=False)

    x_in = nc.declare_dram_parameter("x_in", [IPC, C, H, W], F16, isOutput=False)
    # [K row, image, {side, mid}, M row]
    mats = nc.declare_dram_parameter("mats", [128, IPC, 2, 128], F16, isOutput=False)
    # [halo idx, image, tile, M row]
    hmats = nc.declare_dram_parameter("hmats", [6, IPC, NT, 128], F16, isOutput=False)
    # fp16, scaled by 2**12 to stay out of the subnormal range
    emat = nc.declare_dram_parameter("emat", [128, IPC, 128], F16, isOutput=False)
    gcol = nc.declare_dram_parameter("gcol", [128, IPC], F32, isOutput=False)
    scal = nc.declare_dram_parameter("scal", [128, IPC * C], F32, isOutput=False)
    y_out = nc.declare_dram_parameter("y_out", [IPC, C, H, W], F16, isOutput=True)

    AF = mybir.ActivationFunctionType
    ALU = mybir.AluOpType

    with ExitStack() as ctx:
        tc = ctx.enter_context(tile.TileContext(nc))
        singles = ctx.enter_context(tc.tile_pool(name="singles", bufs=1))
        xp = ctx.enter_context(tc.tile_pool(name="xp", bufs=4))
        lnp = ctx.enter_context(tc.tile_pool(name="lnp", bufs=2))
        tpp = ctx.enter_context(tc.tile_pool(name="tpp", bufs=3))
        hsp = ctx.enter_context(tc.tile_pool(name="hsp", bufs=3))
        w1p = ctx.enter_context(tc.tile_pool(name="w1p", bufs=2))
        up = ctx.enter_context(tc.tile_pool(name="up", bufs=3))
        accp = ctx.enter_context(tc.tile_pool(name="accp", bufs=4))
        cbpp = ctx.enter_context(tc.tile_pool(name="cbpp", bufs=1, space="PSUM"))
        cbp = ctx.enter_context(tc.tile_pool(name="cbp", bufs=4))
        halop = ctx.enter_context(tc.tile_pool(name="halop", bufs=3))
        outpp = ctx.enter_context(tc.tile_pool(name="outpp", bufs=3, space="PSUM"))
        outsp = ctx.enter_context(tc.tile_pool(name="outsp", bufs=2))

        # ---- constants into SBUF (one DMA each, scalar HWDGE ring) ----
        mats_sb = singles.tile([128, IPC, 2, 128], F16)
        nc.scalar.dma_start(out=mats_sb[:, :, :, :], in_=mats[:, :, :, :])
        hmats_sb = singles.tile([6, IPC, NT, 128], F16)
        nc.scalar.dma_start(out=hmats_sb[:, :, :, :], in_=hmats[:, :, :, :])
        emat_sb = singles.tile([128, IPC, 128], F16)
        nc.scalar.dma_start(out=emat_sb[:, :, :], in_=emat[:, :, :])
        gcol_sb = singles.tile([128, IPC], F32)
        nc.scalar.dma_start(out=gcol_sb[:, :], in_=gcol[:, :])
        scal_sb = singles.tile([128, IPC * C], F32)
        nc.scalar.dma_start(out=scal_sb[:, :], in_=scal[:, :])

        # ---- PE HAM warm-up through the pipeline-fill bubble (fp16: one
        # hardware pass per matmul, unlike fp32's LOW_HIGH pair) ----
        wps = cbpp.tile([128, 512], F32, tag="cbps")
        for _ in range(8):
            nc.tensor.matmul(
                out=wps[:, 0:512],
                lhsT=mats_sb[:, 0, 0, :],
                rhs=mats_sb[:, :, :, :].rearrange("p a b m -> p (a b m)")[:, 0:512],
                start=True,
                stop=True,
            )

        # ---- image loads on the scalar HWDGE ring (image 0 split per
        # channel so the first Ln starts ~1.5us in) ----
        xbs = [
            xp.tile([128, C, NT, 512], F16, name=f"xb{i}", tag="xb")
            for i in range(IPC)
        ]

        def load_image(i):
            nc.scalar.dma_start(
                out=xbs[i][:, :, :, :],
                in_=x_in[i].rearrange("c (k p) j -> p c k j", p=128),
            )

        for c_ in range(C):
            nc.scalar.dma_start(
                out=xbs[0][:, c_, :, :],
                in_=x_in[0, c_].rearrange("(k p) j -> p k j", p=128),
            )
        load_image(1)

        lts = {}

        def ln_image(i):
            lts[i] = lnp.tile([128, C, NT, 512], F16, name=f"lt{i}", tag="lt")
            if i == 0:
                for c_ in range(C):
                    nc.scalar.activation(
                        out=lts[i][:, c_, :, :], in_=xbs[i][:, c_, :, :], func=AF.Ln
                    )
            else:
                nc.scalar.activation(
                    out=lts[i][:, :, :, :], in_=xbs[i][:, :, :, :], func=AF.Ln
                )

        ln_image(0)

        ocs = {}

        def phase_a(i, c):
            ch = i * C + c
            acc = accp.tile([128, 1], F32)
            tpc = tpp.tile([128, NT, 512], F16)
            nc.scalar.activation(
                out=tpc[:, :, :],
                in_=lts[i][:, c, :, :],
                func=AF.Exp,
                scale=gcol_sb[:, i : i + 1],
                bias=scal_sb[:, ch : ch + 1],
                accum_out=acc[:, 0:1],
            )
            # ---- mean -> cb column: cb[m] = const * sum_p acc[p] (fp16
            # matmul: fp32 would lower to a LOW_HIGH pair on the PE) ----
            acc16 = accp.tile([128, 1], F16, tag="acc16")
            nc.vector.tensor_copy(out=acc16[:, :], in_=acc[:, :])
            cbps = cbpp.tile([128, 1], F32, tag="cbps")
            nc.tensor.matmul(
                out=cbps[:, :],
                lhsT=emat_sb[:, i, :],
                rhs=acc16[:, :],
                start=True,
                stop=True,
            )
            cb = cbp.tile([128, 1], F32)
            nc.vector.tensor_scalar(
                cb[:, :], cbps[:, :], float(2.0**-12), None, ALU.mult
            )
            return {"i": i, "c": c, "tpc": tpc, "cb": cb}

        def phase_b(st):
            i, c = st["i"], st["c"]
            tpc, cb = st["tpc"], st["cb"]
            # ---- u = clip01(t' + cb) in fp16 (4x mode) ----
            uc = up.tile([128, NT, 512], F16)
            if slotmask[i]:
                # a > 1 -> cb < 0: need the max(.,0)
                w1c = w1p.tile([128, NT, 512], F16)
                nc.vector.tensor_scalar(
                    w1c[:, :, :], tpc[:, :, :], cb[:, 0:1], 0.0, ALU.add, ALU.max
                )
                nc.vector.tensor_scalar(uc[:, :, :], w1c[:, :, :], 1.0, None, ALU.min)
            else:
                # a <= 1 -> cb >= 0 and t' >= 0: max(.,0) is a no-op
                nc.vector.tensor_scalar(
                    uc[:, :, :], tpc[:, :, :], cb[:, 0:1], 1.0, ALU.add, ALU.min
                )

            # ---- hside = left+right neighbor sum (horizontal side taps);
            # tile col t holds image col t-1 so the packed-2x write at
            # [2:512] stays 4-byte aligned ----
            hside = hsp.tile([128, NT, 514], F16)
            nc.vector.tensor_tensor(
                hside[:, :, 2:512], uc[:, :, 0:510], uc[:, :, 2:512], ALU.add
            )
            nc.vector.tensor_copy(
                out=hside[:, :, 1:513:511], in_=uc[:, :, 1:512:509]
            )

            # ---- halo rows: [0:3]=rows{127,255,383}, [3:6]=rows{128,256,384};
            # 3-summed horizontally on GPSIMD ----
            halo = halop.tile([6, 512], F16, name="halo", tag="halo")
            nc.sync.dma_start(out=halo[0:3, :], in_=uc[127:128, 0:3, :])
            nc.sync.dma_start(out=halo[3:6, :], in_=uc[0:1, 1:4, :])
            hpair = halop.tile([6, 512], F16, name="hpair", tag="hpair")
            hs = halop.tile([6, 512], F16, name="hs", tag="hs")
            nc.gpsimd.tensor_tensor(
                hpair[:, 0:511], halo[:, 0:511], halo[:, 1:512], ALU.add
            )
            nc.gpsimd.tensor_tensor(
                hs[:, 1:511], hpair[:, 0:510], halo[:, 2:512], ALU.add
            )
            nc.vector.tensor_copy(out=hs[:, 0:512:511], in_=hpair[:, 0:511:510])

            # ---- conv matmuls + clip into the per-image out tile ----
            if c == 0:
                ocs[i] = outsp.tile([128, C, NT, 512], F16, name=f"oc{i}", tag="oc")
            oc = ocs[i]
            mmid = mats_sb[:, i, 1, :]
            mside = mats_sb[:, i, 0, :]
            for g2 in range(2):
                ob = outpp.tile([128, 2, 512], F32)
                for kk in range(2):
                    nc.tensor.matmul(
                        out=ob[:, kk, 0:512],
                        lhsT=mmid,
                        rhs=uc[:, 2 * g2 + kk, 0:512],
                        start=True,
                        stop=False,
                    )
                for kk in range(2):
                    nc.tensor.matmul(
                        out=ob[:, kk, 0:512],
                        lhsT=mside,
                        rhs=hside[:, 2 * g2 + kk, 1:513],
                        start=False,
                        stop=False,
                    )
                for kk in range(2):
                    nc.tensor.matmul(
                        out=ob[:, kk, 0:512],
                        lhsT=hmats_sb[0:6, i, 2 * g2 + kk, :],
                        rhs=hs[:, 0:512],
                        start=False,
                        stop=True,
                    )
                nc.vector.tensor_scalar(
                    oc[:, c, 2 * g2 : 2 * g2 + 2, :],
                    ob[:, :, :],
                    0.0,
                    1.0,
                    ALU.max,
                    ALU.min,
                )
            if c == C - 1:
                nc.scalar.dma_start(
                    out=y_out[i].rearrange("c (k p) j -> p c k j", p=128),
                    in_=oc[:, :, :, :],
                )
                if i + 2 < IPC:
                    load_image(i + 2)

        chans = [(i, c) for i in range(IPC) for c in range(C)]
        prev = None
        for i, c in chans:
            st = phase_a(i, c)
            if c == 1 and i + 1 < IPC:
                ln_image(i + 1)
            if prev is not None:
                phase_b(prev)
            prev = st
        phase_b(prev)
    nc.compile()
    return nc


def _host_inputs(x, gamma, wb, contrast, sharpen_strength, idx):
    """Build per-core input maps (numpy only). idx[cid][i] = global image."""
    in_maps = []
    for cid in range(NCORES):
        imgs = idx[cid]
        mats = np.zeros((128, IPC, 2, 128), np.float16)
        hmats = np.zeros((6, IPC, NT, 128), np.float16)
        emat = np.zeros((128, IPC, 128), np.float16)
        gcol = np.zeros((128, IPC), np.float32)
        scal = np.zeros((128, IPC * C), np.float32)
        for i in range(IPC):
            b = imgs[i]
            a = float(contrast[b])
            s = float(sharpen_strength[b])
            g = float(gamma[b])
            ns = np.float16(-s)
            c8 = np.float16(1.0 + 8.0 * s)
            # mats[:, i, 0] = Mside (all -s taps), mats[:, i, 1] = Mmid
            # (center 1+8s so PSUM holds u + s*(8u - neighbors) directly)
            for m in range(128):
                for dp_ in (-1, 0, 1):
                    p = m + dp_
                    if 0 <= p < 128:
                        mats[p, i, 0, m] = ns
                        mats[p, i, 1, m] = c8 if dp_ == 0 else ns
            # halo rows {127,128,255,256,383,384}: tile k's top neighbor row
            # 128k-1 is halo idx k-1; bottom neighbor 128k+128 is 3+k
            for k in range(NT):
                if k >= 1:
                    hmats[k - 1, i, k, 0] = ns
                if k <= 2:
                    hmats[3 + k, i, k, 127] = ns
            # scaled by 2**12: raw value ~1e-6 would be fp16-subnormal
            emat[:, i, :] = (1.0 - a) / (a * NPIX) * 4096.0
            gcol[:, i] = g
            for c in range(C):
                scal[:, i * C + c] = np.log(a * float(wb[b, c]))
        in_maps.append(
            {
                "x_in": np.ascontiguousarray(x[imgs]).astype(np.float16),
                "mats": mats,
                "hmats": hmats,
                "emat": emat,
                "gcol": gcol,
                "scal": scal,
            }
        )
    return in_maps


_PROGRAM_CACHE = {}


def kernel(x, gamma, wb, contrast, sharpen_strength):
    x = np.asarray(x, dtype=np.float32)
    gamma = np.asarray(gamma, dtype=np.float32)
    wb = np.asarray(wb, dtype=np.float32)
    contrast = np.asarray(contrast, dtype=np.float32)
    sharpen_strength = np.asarray(sharpen_strength, dtype=np.float32)

    # Sort images by contrast and stripe across cores so slot i is
    # homogeneous in sign(1-a); the single-op clip path is only legal
    # when every image in the slot has a <= 1 (SPMD: shared program).
    order = np.argsort(contrast, kind="stable")
    idx = [[int(order[i * NCORES + cid]) for i in range(IPC)] for cid in range(NCORES)]
    slotmask = tuple(
        bool(any(contrast[order[i * NCORES + cid]] > 1.0 for cid in range(NCORES)))
        for i in range(IPC)
    )
    if slotmask not in _PROGRAM_CACHE:
        _PROGRAM_CACHE.clear()
        _PROGRAM_CACHE[slotmask] = _build_program(slotmask)
    nc = _PROGRAM_CACHE[slotmask]

    in_maps = _host_inputs(x, gamma, wb, contrast, sharpen_strength, idx)
    res = run_bass_kernel_spmd(nc, in_maps, list(range(NCORES)))
    out = np.empty((B, C, H, W), np.float32)
    for cid in range(NCORES):
        for i in range(IPC):
            out[idx[cid][i]] = res.results[cid]["y_out"][i].astype(np.float32)
    return out


# revision 27
# speedup vs baseline: 1.0361x; 1.0361x over previous
"""Trainium2 Bass kernel for the DIP module (tone curve + white balance +
contrast-about-mean + 3x3 sharpen blend), data-parallel over batch on 8 cores.

v2 — fp16 end-to-end. Pipeline per (image, channel), channel = [128p, 4k, 512]:
  lt = Ln(x)                 ACT, one instr per IMAGE (FD=6144), fp16 in/out
  t' = a*wb * x^g            ACT Exp(scale=g, bias=ln(a*wb)) -> fp16,
                             accum_out = per-row sums (for the mean)
  cb = mean(t')*(1-a)/a      one fp32 matmul (const matrix) + DVE copy
  u  = clip01(t' + cb)       DVE tensor_scalar fp16->fp16 (4x mode); 1 op when
                             a<=1 (images permuted so slots sign-homogeneous)
  y  = clip01((1+8s)u - s*(8-neighbors))
                             4 fp16 matmuls per 128-row tile into PSUM with the
                             (1+8s) center folded into the mid matrix; halo rows
                             gathered by 2 DMAs, 3-summed on GPSIMD; final clip
                             reads 2-bank PSUM groups -> fp16 out, one DMA/image

I/O is fp16 in DRAM (host casts f32->fp16 on the way in, fp16->f32 out); loads
ride the sync HWDGE ring, stores + const the scalar ring, halo gathers on sync.
"""

import numpy as np

try:
    import concourse.bass as bass
except ImportError:  # pragma: no cover
    import sys

    sys.path.insert(0, "/opt/trn_rl_repo")
    import concourse.bass as bass

from contextlib import ExitStack

import concourse.bacc as bacc
import concourse.tile as tile
from concourse import mybir
from concourse.bass_utils import run_bass_kernel_spmd

F32 = mybir.dt.float32
F16 = mybir.dt.float16

B, C, H, W = 32, 3, 512, 512
NCORES = 8
IPC = B // NCORES  # images per core
NT = H // 128  # row tiles per channel
NPIX = H * W


class _Bacc(bacc.Bacc):
    """Bacc that pins Exp/Ln to the combined table set so the kernel does a
    single ACT_TABLE_LOAD instead of thrashing between exp/ln sets."""

    def insert_act_table_loads(self):
        import bass_rust as _bass_rust

        from concourse.hw_specs import get_activation_tables

        has_activation = any(
            isinstance(i, mybir.InstActivation)
            for b in self.main_func.blocks
            for i in b.instructions
        )
        if not has_activation:
            return
        AF = mybir.ActivationFunctionType
        tables = []
        for name, funcs in get_activation_tables(self.m.arch).items():
            if name != "natural_log_exp_and_others":
                funcs = funcs - {AF.Exp, AF.Ln}
            tables.append((name, funcs))
        _bass_rust.insert_act_table_loads(self, tables)


def _build_program(slotmask):
    nc = _Bacc("TRN2", target_bir_lowering=False)

    x_in = nc.declare_dram_parameter("x_in", [IPC, C, H, W], F16, isOutput=False)
    # [K row, image, {side, mid}, M row]
    mats = nc.declare_dram_parameter("mats", [128, IPC, 2, 128], F16, isOutput=False)
    # [halo idx, image, tile, M row]
    hmats = nc.declare_dram_parameter("hmats", [6, IPC, NT, 128], F16, isOutput=False)
    # fp16, scaled by 2**12 to stay out of the subnormal range
    emat = nc.declare_dram_parameter("emat", [128, IPC, 128], F16, isOutput=False)
    gcol = nc.declare_dram_parameter("gcol", [128, IPC], F32, isOutput=False)
    scal = nc.declare_dram_parameter("scal", [128, IPC * C], F32, isOutput=False)
    y_out = nc.declare_dram_parameter("y_out", [IPC, C, H, W], F16, isOutput=True)

    AF = mybir.ActivationFunctionType
    ALU = mybir.AluOpType

    with ExitStack() as ctx:
        tc = ctx.enter_context(tile.TileContext(nc))
        singles = ctx.enter_context(tc.tile_pool(name="singles", bufs=1))
        xp = ctx.enter_context(tc.tile_pool(name="xp", bufs=4))
        lnp = ctx.enter_context(tc.tile_pool(name="lnp", bufs=2))
        tpp = ctx.enter_context(tc.tile_pool(name="tpp", bufs=3))
        hsp = ctx.enter_context(tc.tile_pool(name="hsp", bufs=3))
        w1p = ctx.enter_context(tc.tile_pool(name="w1p", bufs=2))
        up = ctx.enter_context(tc.tile_pool(name="up", bufs=3))
        accp = ctx.enter_context(tc.tile_pool(name="accp", bufs=4))
        cbpp = ctx.enter_context(tc.tile_pool(name="cbpp", bufs=1, space="PSUM"))
        cbp = ctx.enter_context(tc.tile_pool(name="cbp", bufs=4))
        halop = ctx.enter_context(tc.tile_pool(name="halop", bufs=3))
        outpp = ctx.enter_context(tc.tile_pool(name="outpp", bufs=3, space="PSUM"))
        outsp = ctx.enter_context(tc.tile_pool(name="outsp", bufs=2))

        # ---- constants into SBUF (one DMA each, scalar HWDGE ring) ----
        mats_sb = singles.tile([128, IPC, 2, 128], F16)
        nc.scalar.dma_start(out=mats_sb[:, :, :, :], in_=mats[:, :, :, :])
        hmats_sb = singles.tile([6, IPC, NT, 128], F16)
        nc.scalar.dma_start(out=hmats_sb[:, :, :, :], in_=hmats[:, :, :, :])
        emat_sb = singles.tile([128, IPC, 128], F16)
        nc.scalar.dma_start(out=emat_sb[:, :, :], in_=emat[:, :, :])
        gcol_sb = singles.tile([128, IPC], F32)
        nc.scalar.dma_start(out=gcol_sb[:, :], in_=gcol[:, :])
        scal_sb = singles.tile([128, IPC * C], F32)
        nc.scalar.dma_start(out=scal_sb[:, :], in_=scal[:, :])

        # ---- PE HAM warm-up through the pipeline-fill bubble (fp16: one
        # hardware pass per matmul, unlike fp32's LOW_HIGH pair) ----
        wps = cbpp.tile([128, 512], F32, tag="cbps")
        for _ in range(8):
            nc.tensor.matmul(
                out=wps[:, 0:512],
                lhsT=mats_sb[:, 0, 0, :],
                rhs=mats_sb[:, :, :, :].rearrange("p a b m -> p (a b m)")[:, 0:512],
                start=True,
                stop=True,
            )

        # ---- image loads on the scalar HWDGE ring (image 0 split per
        # channel so the first Ln starts ~1.5us in) ----
        xbs = [
            xp.tile([128, C, NT, 512], F16, name=f"xb{i}", tag="xb")
            for i in range(IPC)
        ]

        def load_image(i):
            nc.scalar.dma_start(
                out=xbs[i][:, :, :, :],
                in_=x_in[i].rearrange("c (k p) j -> p c k j", p=128),
            )

        for c_ in range(C):
            nc.scalar.dma_start(
                out=xbs[0][:, c_, :, :],
                in_=x_in[0, c_].rearrange("(k p) j -> p k j", p=128),
            )
        load_image(1)

        lts = {}

        def ln_image(i):
            lts[i] = lnp.tile([128, C, NT, 512], F16, name=f"lt{i}", tag="lt")
            if i == 0:
                for c_ in range(C):
                    nc.scalar.activation(
                        out=lts[i][:, c_, :, :], in_=xbs[i][:, c_, :, :], func=AF.Ln
                    )
            else:
                nc.scalar.activation(
                    out=lts[i][:, :, :, :], in_=xbs[i][:, :, :, :], func=AF.Ln
                )

        ln_image(0)

        ocs = {}

        def phase_a(i, c):
            ch = i * C + c
            acc = accp.tile([128, 1], F32)
            tpc = tpp.tile([128, NT, 512], F16)
            nc.scalar.activation(
                out=tpc[:, :, :],
                in_=lts[i][:, c, :, :],
                func=AF.Exp,
                scale=gcol_sb[:, i : i + 1],
                bias=scal_sb[:, ch : ch + 1],
                accum_out=acc[:, 0:1],
            )
            # ---- mean -> cb column: cb[m] = const * sum_p acc[p] (fp16
            # matmul: fp32 would lower to a LOW_HIGH pair on the PE) ----
            acc16 = accp.tile([128, 1], F16, tag="acc16")
            nc.vector.tensor_copy(out=acc16[:, :], in_=acc[:, :])
            cbps = cbpp.tile([128, 1], F32, tag="cbps")
            nc.tensor.matmul(
                out=cbps[:, :],
                lhsT=emat_sb[:, i, :],
                rhs=acc16[:, :],
                start=True,
                stop=True,
            )
            cb = cbp.tile([128, 1], F32)
            nc.vector.tensor_scalar(
                cb[:, :], cbps[:, :], float(2.0**-12), None, ALU.mult
            )
            return {"i": i, "c": c, "tpc": tpc, "cb": cb}

        def phase_b(st):
            i, c = st["i"], st["c"]
            tpc, cb = st["tpc"], st["cb"]
            # ---- u = clip01(t' + cb) in fp16 (4x mode) ----
            uc = up.tile([128, NT, 512], F16)
            if slotmask[i]:
                # a > 1 -> cb < 0: need the max(.,0)
                w1c = w1p.tile([128, NT, 512], F16)
                nc.vector.tensor_scalar(
                    w1c[:, :, :], tpc[:, :, :], cb[:, 0:1], 0.0, ALU.add, ALU.max
                )
                nc.vector.tensor_scalar(uc[:, :, :], w1c[:, :, :], 1.0, None, ALU.min)
            else:
                # a <= 1 -> cb >= 0 and t' >= 0: max(.,0) is a no-op
                nc.vector.tensor_scalar(
                    uc[:, :, :], tpc[:, :, :], cb[:, 0:1], 1.0, ALU.add, ALU.min
                )

            # ---- hside = left+right neighbor sum (horizontal side taps);
            # tile col t holds image col t-1 so the packed-2x write at
            # [2:512] stays 4-byte aligned ----
            hside = hsp.tile([128, NT, 514], F16)
            nc.vector.tensor_tensor(
                hside[:, :, 2:512], uc[:, :, 0:510], uc[:, :, 2:512], ALU.add
            )
            nc.vector.tensor_copy(
                out=hside[:, :, 1:513:511], in_=uc[:, :, 1:512:509]
            )

            # ---- halo rows: [0:3]=rows{127,255,383}, [3:6]=rows{128,256,384};
            # 3-summed horizontally on GPSIMD ----
            halo = halop.tile([6, 512], F16, name="halo", tag="halo")
            nc.sync.dma_start(out=halo[0:3, :], in_=uc[127:128, 0:3, :])
            nc.sync.dma_start(out=halo[3:6, :], in_=uc[0:1, 1:4, :])
            hpair = halop.tile([6, 512], F16, name="hpair", tag="hpair")
            hs = halop.tile([6, 512], F16, name="hs", tag="hs")
            nc.vector.tensor_tensor(
                hpair[:, 0:511], halo[:, 0:511], halo[:, 1:512], ALU.add
            )
            nc.gpsimd.tensor_tensor(
                hs[:, 1:511], hpair[:, 0:510], halo[:, 2:512], ALU.add
            )
            nc.vector.tensor_copy(out=hs[:, 0:512:511], in_=hpair[:, 0:511:510])

            # ---- conv matmuls + clip into the per-image out tile ----
            if c == 0:
                ocs[i] = outsp.tile([128, C, NT, 512], F16, name=f"oc{i}", tag="oc")
            oc = ocs[i]
            mmid = mats_sb[:, i, 1, :]
            mside = mats_sb[:, i, 0, :]
            # mid+side matmuls of BOTH psum groups first, halos last: the
            # halo rows come through a gather->3-sum chain that this order
            # hides under ~8 matmuls of PE work
            obs = [
                outpp.tile([128, 2, 512], F32, name=f"ob{g2}", tag="ob")
                for g2 in range(2)
            ]
            for g2 in range(2):
                for kk in range(2):
                    nc.tensor.matmul(
                        out=obs[g2][:, kk, 0:512],
                        lhsT=mmid,
                        rhs=uc[:, 2 * g2 + kk, 0:512],
                        start=True,
                        stop=False,
                    )
                for kk in range(2):
                    nc.tensor.matmul(
                        out=obs[g2][:, kk, 0:512],
                        lhsT=mside,
                        rhs=hside[:, 2 * g2 + kk, 1:513],
                        start=False,
                        stop=False,
                    )
            for g2 in range(2):
                for kk in range(2):
                    nc.tensor.matmul(
                        out=obs[g2][:, kk, 0:512],
                        lhsT=hmats_sb[0:6, i, 2 * g2 + kk, :],
                        rhs=hs[:, 0:512],
                        start=False,
                        stop=True,
                    )
                nc.vector.tensor_scalar(
                    oc[:, c, 2 * g2 : 2 * g2 + 2, :],
                    obs[g2][:, :, :],
                    0.0,
                    1.0,
                    ALU.max,
                    ALU.min,
                )
            if c == C - 1:
                nc.scalar.dma_start(
                    out=y_out[i].rearrange("c (k p) j -> p c k j", p=128),
                    in_=oc[:, :, :, :],
                )
                if i + 2 < IPC:
                    load_image(i + 2)

        chans = [(i, c) for i in range(IPC) for c in range(C)]
        prev = None
        for i, c in chans:
            st = phase_a(i, c)
            if c == 1 and i + 1 < IPC:
                ln_image(i + 1)
            if prev is not None:
                phase_b(prev)
            prev = st
        phase_b(prev)
    nc.compile()
    return nc


def _host_inputs(x, gamma, wb, contrast, sharpen_strength, idx):
    """Build per-core input maps (numpy only). idx[cid][i] = global image."""
    in_maps = []
    for cid in range(NCORES):
        imgs = idx[cid]
        mats = np.zeros((128, IPC, 2, 128), np.float16)
        hmats = np.zeros((6, IPC, NT, 128), np.float16)
        emat = np.zeros((128, IPC, 128), np.float16)
        gcol = np.zeros((128, IPC), np.float32)
        scal = np.zeros((128, IPC * C), np.float32)
        for i in range(IPC):
            b = imgs[i]
            a = float(contrast[b])
            s = float(sharpen_strength[b])
            g = float(gamma[b])
            ns = np.float16(-s)
            c8 = np.float16(1.0 + 8.0 * s)
            # mats[:, i, 0] = Mside (all -s taps), mats[:, i, 1] = Mmid
            # (center 1+8s so PSUM holds u + s*(8u - neighbors) directly)
            for m in range(128):
                for dp_ in (-1, 0, 1):
                    p = m + dp_
                    if 0 <= p < 128:
                        mats[p, i, 0, m] = ns
                        mats[p, i, 1, m] = c8 if dp_ == 0 else ns
            # halo rows {127,128,255,256,383,384}: tile k's top neighbor row
            # 128k-1 is halo idx k-1; bottom neighbor 128k+128 is 3+k
            for k in range(NT):
                if k >= 1:
                    hmats[k - 1, i, k, 0] = ns
                if k <= 2:
                    hmats[3 + k, i, k, 127] = ns
            # scaled by 2**12: raw value ~1e-6 would be fp16-subnormal
            emat[:, i, :] = (1.0 - a) / (a * NPIX) * 4096.0
            gcol[:, i] = g
            for c in range(C):
                scal[:, i * C + c] = np.log(a * float(wb[b, c]))
        in_maps.append(
            {
                "x_in": np.ascontiguousarray(x[imgs]).astype(np.float16),
                "mats": mats,
                "hmats": hmats,
                "emat": emat,
                "gcol": gcol,
                "scal": scal,
            }
        )
    return in_maps


_PROGRAM_CACHE = {}


def kernel(x, gamma, wb, contrast, sharpen_strength):
    x = np.asarray(x, dtype=np.float32)
    gamma = np.asarray(gamma, dtype=np.float32)
    wb = np.asarray(wb, dtype=np.float32)
    contrast = np.asarray(contrast, dtype=np.float32)
    sharpen_strength = np.asarray(sharpen_strength, dtype=np.float32)

    # Sort images by contrast and stripe across cores so slot i is
    # homogeneous in sign(1-a); the single-op clip path is only legal
    # when every image in the slot has a <= 1 (SPMD: shared program).
    order = np.argsort(contrast, kind="stable")
    idx = [[int(order[i * NCORES + cid]) for i in range(IPC)] for cid in range(NCORES)]
    slotmask = tuple(
        bool(any(contrast[order[i * NCORES + cid]] > 1.0 for cid in range(NCORES)))
        for i in range(IPC)
    )
    if slotmask not in _PROGRAM_CACHE:
        _PROGRAM_CACHE.clear()
        _PROGRAM_CACHE[slotmask] = _build_program(slotmask)
    nc = _PROGRAM_CACHE[slotmask]

    in_maps = _host_inputs(x, gamma, wb, contrast, sharpen_strength, idx)
    res = run_bass_kernel_spmd(nc, in_maps, list(range(NCORES)))
    out = np.empty((B, C, H, W), np.float32)
    for cid in range(NCORES):
        for i in range(IPC):
            out[idx[cid][i]] = res.results[cid]["y_out"][i].astype(np.float32)
    return out
